# revision 45
# baseline (speedup 1.0000x reference)
"""Trainium2 Bass kernel for nn_CrossAttentionLayer (sparse cross attention).

Sharding: 8 cores = 4 batches x 2 head-groups. Core c handles batch c//2 and
heads [4*(c%2), 4*(c%2)+4). Host compacts the kv sequence per batch using
kv_mask, padding to a multiple of 384 tokens.

v3 design (cost-model-driven):
  - Scores run as TWO fp8 DoubleRow matmuls per (kv-tile, 512q) tile with a
    hi/lo split for full bf16-level precision at half the bf16 row cost:
      DR-A: k_hi.T q_hi + k_hi.T q_lo   (stride-0 weight slot pair)
      DR-B: k_lo.T q_hi + I.T M
    where x = fp8(x_hi) + fp8(x_lo) reconstructs x to ~0.2% and M holds the
    sparse-mask bias {+112 valid, -128 masked} plus the softmax range shift.
    The k_lo*q_lo term (~0.1%) is dropped.  No elementwise mask work exists
    anywhere.
  - p tiles and v are fp16.  exp splits across three engines: ACT (true exp,
    bias matching the Schraudolph scale), DVE and Pool (Schraudolph:
    int16 bits = floor(max(128*log2e*(s+112), 0)) bitcast fp16; masked
    entries clamp to exactly +0.0).
  - q/k/v/out projections and attn@V stay bf16/fp16 for accuracy.
  - All layernorm transposes go through DMA-transpose (no PSUM round trip);
    LN applies use DVE's 4x mode.  kproj uses per-head zero-interleaved
    weights so kT hi/lo slots get PE-computed zero padding in the
    complementary head half.
"""

import math
import os

import numpy as np
import ml_dtypes

import bass_rust
import concourse.bass as bass
import concourse.mybir as mybir
import concourse.tile as tile
from concourse import bass_utils
from concourse.ap import AP
from concourse.masks import make_identity
from concourse.vector_clock import ScopedClock


class _TileContext(tile.TileContext):
    """TileContext whose kernel-tail drain is split into single-wait drains.

    The walrus build in this environment rejects >1 sync-wait on a Drain
    (CTRL_NO struct): "Too many sync wait commands". The stock
    _drain_and_barrier attaches one wait per outstanding semaphore to a
    single Drain; emit one Drain per wait instead.
    """

    def _drain_and_barrier(self, tick_clock, wait_clock):
        drain_inst = self.nc.sync.drain()
        wait_clock.add_sem_waits(
            drain_inst.ins, ScopedClock({None: tick_clock.global_clock})
        )
        si = drain_inst.ins.sync_info
        if si is not None and si.on_wait and len(si.on_wait) > 1:
            waits = list(si.on_wait)
            drain_inst.ins.sync_info = bass_rust.SyncInfo(
                on_wait=[waits[0]], on_update=si.on_update or [])
            for w in waits[1:]:
                extra = self.nc.sync.drain()
                extra.ins.sync_info = bass_rust.SyncInfo(
                    on_wait=[w], on_update=[])

        self.nc.all_engine_barrier()
        assert self.sems is not None
        popped = self.nc._tile_sem_poison_stack.pop()
        assert popped is self._sem_poison
        self.nc.clear_and_free_semaphores(list(self.sems.allocated().values()))
        self.nc.all_engine_barrier()


def _split_sync_waits(nc):
    """Cap every instruction at one sync wait (walrus build limitation)."""
    for f in nc.m.functions:
        for bb in f.blocks:
            insns = bb.instructions
            out = []
            changed = False
            for ins in insns:
                si = ins.sync_info
                if si is not None and si.on_wait and len(si.on_wait) > 1:
                    waits = list(si.on_wait)
                    for w in waits[:-1]:
                        nop = mybir.InstNoOp(
                            name=nc.get_next_instruction_name(),
                            engine=ins.engine,
                            ins=[], outs=[],
                            sync_info=bass_rust.SyncInfo(
                                on_wait=[w], on_update=[]),
                        )
                        out.append(nop)
                    ins.sync_info = bass_rust.SyncInfo(
                        on_wait=[waits[-1]], on_update=si.on_update or [])
                    changed = True
                out.append(ins)
            if changed:
                bb.instructions = out


BF16 = ml_dtypes.bfloat16
F8 = ml_dtypes.float8_e4m3
FP16 = np.float16

E = 512
H = 8
D = 64
T = 2048           # query tokens
P = 128
NQT = T // P       # 16 query token tiles
EC = E // P        # 4 contraction chunks
HC = 4             # heads per core
MC = 2             # 128-wide chunks of this core's 256 head dims
QC = 4             # 512-wide query chunks
SCALE = float(D) ** -0.5
EPS = 1e-5
MASK_ON = 112.0      # valid-entry score bias (fp8-exact)
MASK_OFF = -128.0    # masked-entry score bias (fp8-exact)
# Schraudolph exp in fp16: bits = A16*(s + 112), value = 2^((bits-15360)/1024)
A16 = 184.66496280150618          # 128 * log2(e)
# ACT path must match the Schraudolph path's MEAN scale: the Schraudolph
# chord (1+f)/2^f overestimates by x1.04068 on average, so the ACT path is
# scaled up by the same factor (uniform across chunks -> cancels in softmax):
# exp(SCALE*(s+112) + BIAS_ACT) = 1.04068 * 2^((A16*(s+112) - 15360)/1024)
BIAS_ACT = -10.35732923219427    # (A16*112-15360)*ln2/1024 - 14 + ln(1.04068)

_CACHE = {}
_LAST_KEY = None


def _build(nkt: int, needs_bqk: bool, needs_bv: bool):
    assert nkt % 3 == 0, "kv tiles padded to a multiple of 3"
    nkg = nkt // 3               # kv groups of 3 tiles / score chunks
    KT = nkt * P
    NS = nkt * HC                # hi slots; lo slots at NS+s; identity at 2NS

    nc = bass.Bass("TRN2", target_bir_lowering=False, debug=False,
                   num_devices=8)
    f32 = mybir.dt.float32
    bf16 = mybir.dt.bfloat16
    fp16 = mybir.dt.float16
    fp8 = mybir.dt.float8e4
    i16 = mybir.dt.int16

    xq = nc.dram_tensor("xq", [T, E], bf16, kind="ExternalInput").ap()
    xkv = nc.dram_tensor("xkv", [KT, E], bf16, kind="ExternalInput").ap()
    wq = nc.dram_tensor("wq", [E, MC * P], bf16, kind="ExternalInput").ap()
    # zero-interleaved per-head k weights: [E, HC, P] with head h occupying
    # columns (h%2)*64..(h%2)*64+64, zeros elsewhere
    wkz = nc.dram_tensor("wkz", [E, HC, P], bf16, kind="ExternalInput").ap()
    wv = nc.dram_tensor("wv", [E, MC * P], bf16, kind="ExternalInput").ap()
    wo = nc.dram_tensor("wo", [MC * P, E], bf16, kind="ExternalInput").ap()
    mtd = nc.dram_tensor("mt", [KT, T], fp8, kind="ExternalInput").ap()
    if needs_bqk:
        bqd = nc.dram_tensor("bq", [P, MC], f32, kind="ExternalInput").ap()
        bkd = nc.dram_tensor("bk", [1, HC * P], bf16,
                             kind="ExternalInput").ap()
    if needs_bv:
        bvd = nc.dram_tensor("bv", [1, MC * P], bf16, kind="ExternalInput").ap()
    outd = nc.dram_tensor("out", [T, E], bf16, kind="ExternalOutput").ap()

    mtr = mtd.rearrange("(c p) q -> p c q", p=P)

    with _TileContext(nc) as tc:
        with (
            tc.tile_pool(name="persist", bufs=1) as pp,
            tc.tile_pool(name="xln", bufs=4) as xlnp,
            tc.tile_pool(name="scratch", bufs=4) as scr,
            tc.tile_pool(name="ostage", bufs=2) as outp,
            tc.tile_pool(name="pt", bufs=2) as pTp,
            tc.tile_pool(name="av", bufs=2) as avp,
            tc.tile_pool(name="at", bufs=2) as aTp,
            tc.tile_pool(name="psS", bufs=3, space="PSUM") as psS,
            tc.tile_pool(name="psA", bufs=1, space="PSUM") as psA,
            tc.tile_pool(name="psO", bufs=1, space="PSUM") as psO,
        ):
            # ---- persistent SBUF tensors ----
            xq_sb = pp.tile([P, NQT, E], bf16, tag="xqs")
            xkv_sb = pp.tile([P, nkt, E], bf16, tag="xkvs")
            wq_sb = pp.tile([P, EC, MC * P], bf16, tag="wq")
            wkz_sb = pp.tile([P, EC, HC, P], bf16, tag="wkz")
            wv_sb = pp.tile([P, EC, MC * P], bf16, tag="wv")
            wo_sb = pp.tile([P, MC, E], bf16, tag="wo")
            # kT: hi slots [0, NS), lo slots [NS, 2NS), identity at 2NS
            kt = pp.tile([P, 2 * NS + 1, P], fp8, tag="kt")
            # v: nkt tiles x HC heads x (D + ones-col), fp16
            vt = pp.tile([P, nkt, HC * (D + 1)], fp16, tag="vt")
            # scores rhs: slots [q_hi mc0, q_hi mc1, q_lo mc0, q_lo mc1,
            #                    M_0..M_{nkt-1}]
            S_g = [pp.tile([P, 4 + nkt, 512], fp8, tag=f"S{g}", name=f"S{g}")
                   for g in range(QC)]
            xlnkvT_g = [pp.tile([P, EC, 3 * P], bf16, tag=f"xlnkv{i}",
                                name=f"xlnkv{i}") for i in range(nkg)]
            xlnqT_g = [pp.tile([P, EC, 4 * P], bf16, tag=f"xlnq{g}",
                               name=f"xlnq{g}") for g in range(QC)]
            if needs_bqk:
                bq_sb = pp.tile([P, MC], f32, tag="bq")
                bk_sb = pp.tile([1, HC * P], bf16, tag="bk")
            if needs_bv:
                bv_sb = pp.tile([1, MC * P], bf16, tag="bv")
            if needs_bqk or needs_bv:
                ones1 = pp.tile([1, E], bf16, tag="ones1")
                nc.vector.memset(ones1[:], 1.0)

            eps_sb = pp.tile([P, 1], f32, tag="eps")
            nc.vector.memset(eps_sb[:], EPS)
            bact_sb = pp.tile([P, 1], f32, tag="bact")
            nc.vector.memset(bact_sb[:], BIAS_ACT)
            if needs_bv:
                nc.sync.dma_start(bv_sb[:], bvd)
            make_identity(nc, kt[:, 2 * NS, :])
            ident_bf = pp.tile([P, P], bf16, tag="identbf")
            make_identity(nc, ident_bf[:])
            # ones columns of v (per head, per kv tile)
            vt4 = vt[:].rearrange("p k (h d) -> p k h d", d=D + 1)
            nc.gpsimd.memset(vt4[:, :, :, D], 1.0)

            def pair_ap(base, stride_slots, width):
                """[P, 2, width] AP: slot pair {base, base+stride}."""
                return AP(base.tensor, base.offset,
                          [list(base.ap[0]), [stride_slots * width, 2],
                           [1, width]])

            def ln_s1(xt):
                """LN stage 1: stats + sqrt on a staged [P, E] slice."""
                stats = scr.tile([P, 6], f32, tag="bnstats")
                mv = scr.tile([P, 2], f32, tag="bnmv")
                nc.vector.bn_stats(stats[:], xt)
                nc.vector.bn_aggr(mv[:], stats[:])
                sig = scr.tile([P, 1], f32, tag="sig")
                nc.scalar.activation(
                    sig[:], mv[:, 1:2],
                    mybir.ActivationFunctionType.Sqrt, bias=eps_sb[:])
                return xt, mv, sig

            def ln_apply(st, pool=False):
                xt, mv, sig = st
                rsig = scr.tile([P, 1], f32, tag="rsig")
                nc.vector.reciprocal(rsig[:], sig[:])
                xln = xlnp.tile([P, E], bf16, tag="xln")
                if pool:
                    nc.gpsimd.tensor_scalar(
                        xln[:], xt, mv[:, 0:1], rsig[:],
                        mybir.AluOpType.subtract, mybir.AluOpType.mult)
                else:
                    nc.vector.tensor_scalar(
                        xln[:], xt, mv[:, 0:1], rsig[:],
                        mybir.AluOpType.subtract, mybir.AluOpType.mult)
                return xln

            def ln_s2(st, dstT, ti, pool=False):
                """q LN stage 2: normalize + SP DMA transpose."""
                xln = ln_apply(st, pool)
                nc.sync.dma_start_transpose(
                    dstT[:, :, ti * P:(ti + 1) * P], xln[:])

            def ln_s2_kv(st, dstT, ti):
                """kv LN stage 2: normalize + PE transpose + ACT copy."""
                xln = ln_apply(st)
                tps = psA.tile([P, 4, D + 1], f32, tag="acc")
                tp = tps[:].rearrange("p a b -> p (a b)").bitcast(bf16)
                for c in range(EC):
                    nc.tensor.transpose(
                        tp[:, c * P:(c + 1) * P], xln[:, c * P:(c + 1) * P],
                        ident_bf[:])
                nc.scalar.activation(
                    dstT[:, :, ti * P:(ti + 1) * P],
                    tp[:, 0:E].rearrange("p (c n) -> p c n", n=P),
                    mybir.ActivationFunctionType.Identity)

            def kproj_group(i):
                """Per-head k projection (zero-interleaved weights) + hi/lo
                fp8 writes into kt slots."""
                for h in range(HC):
                    ps = psS.tile([P, 2, 512], f32, tag="sp")
                    pk = ps[:].rearrange("p a b -> p (a b)")[:, 0:3 * P]
                    for c in range(EC):
                        nc.tensor.matmul(
                            pk,
                            lhsT=wkz_sb[:, c, h, :],
                            rhs=xlnkvT_g[i][:, c, :],
                            start=(c == 0),
                            stop=(c == EC - 1 and not needs_bqk))
                    if needs_bqk:
                        nc.tensor.matmul(
                            pk, lhsT=bk_sb[:, h * P:(h + 1) * P],
                            rhs=ones1[:, 0:3 * P], start=False, stop=True)
                    s0 = i * 3 * HC + h
                    bh = kt[:, s0, :]
                    bl = kt[:, NS + s0, :]
                    hi3 = AP(bh.tensor, bh.offset,
                             [list(bh.ap[0]), [HC * P, 3], [1, P]])
                    lo3 = AP(bl.tensor, bl.offset,
                             [list(bl.ap[0]), [HC * P, 3], [1, P]])
                    # ACT stages psum->bf16 (frees the PSUM buf fast);
                    # DVE/Pool split hi = fp8(k), lo = fp8(k - hi) in SBUF.
                    kstg = scr.tile([P, 3, P], bf16, tag="kstg")
                    nc.scalar.activation(
                        kstg[:], pk.rearrange("p (a n) -> p a n", n=P),
                        mybir.ActivationFunctionType.Identity)
                    nc.vector.tensor_copy(hi3, kstg[:])
                    nc.gpsimd.tensor_tensor(
                        lo3, kstg[:], hi3, mybir.AluOpType.subtract)

            def vproj_group(i):
                for ti in range(3):
                    ps = psS.tile([P, 2, 512], f32, tag="sp")
                    pv = ps[:].rearrange("p a b -> p (a b)")[:, 0:HC * D]
                    for c in range(EC):
                        nc.tensor.matmul(
                            pv,
                            lhsT=xlnkvT_g[i][:, c, ti * P:(ti + 1) * P],
                            rhs=wv_sb[:, c, :],
                            start=(c == 0),
                            stop=(c == EC - 1 and not needs_bv))
                    if needs_bv:
                        nc.tensor.matmul(
                            pv, lhsT=ones1[:, 0:P], rhs=bv_sb[:],
                            start=False, stop=True)
                    vd = vt[:, 3 * i + ti].rearrange(
                        "p (h d) -> p h d", d=D + 1)
                    nc.scalar.activation(
                        vd[:, :, 0:D],
                        pv.rearrange("p (h d) -> p h d", d=D),
                        mybir.ActivationFunctionType.Identity)

            def qproj_mc(g, mc, late=False):
                if late:
                    psq = psO.tile([P, E], f32, tag="po", name="psq")[:]
                else:
                    psq = psS.tile([P, 2, 512], f32, tag="sp",
                                   name="psq")[:, 0, :]
                for c in range(EC):
                    nc.tensor.matmul(
                        psq,
                        lhsT=wq_sb[:, c, mc * P:(mc + 1) * P],
                        rhs=xlnqT_g[g][:, c, :],
                        start=(c == 0), stop=(c == EC - 1))
                if late and not needs_bqk:
                    # block phase: DVE stages, Pool (idle there) splits
                    qstg = scr.tile([P, E], bf16, tag="qstg")
                    nc.vector.tensor_copy(qstg[:], psq)
                    nc.gpsimd.tensor_copy(S_g[g][:, mc, :], qstg[:])
                    nc.gpsimd.tensor_tensor(
                        S_g[g][:, 2 + mc, :], qstg[:], S_g[g][:, mc, :],
                        mybir.AluOpType.subtract)
                else:
                    # prologue: ACT stages psum->bf16 (with bias), then
                    # DVE hi + Pool lo split in SBUF
                    qstg = scr.tile([P, E], bf16, tag="qstg")
                    nc.scalar.activation(
                        qstg[:], psq,
                        mybir.ActivationFunctionType.Identity,
                        bias=bq_sb[:, mc:mc + 1] if needs_bqk else 0.0)
                    nc.vector.tensor_copy(S_g[g][:, mc, :], qstg[:])
                    nc.gpsimd.tensor_tensor(
                        S_g[g][:, 2 + mc, :], qstg[:], S_g[g][:, mc, :],
                        mybir.AluOpType.subtract)

            # ---- prologue: kv side then q groups 0-1, 2-stage pipelined ----
            kv_entries = [("kv", xkv, t) for t in range(3 * nkg)]
            q_entries = [("q", xq, t) for t in range(6)]
            seq = []
            qi = 0
            for t in range(3 * nkg):
                seq.append(kv_entries[t])
                if t % 3 == 2 and qi < 2:
                    seq.append(q_entries[qi])
                    qi += 1
            seq += q_entries[qi:]
            xkvr = xkv.rearrange("(t p) e -> p t e", p=P)
            xqr = xq.rearrange("(t p) e -> p t e", p=P)
            nc.sync.dma_start(xkv_sb[:, 0:1, :], xkvr[:, 0:1, :])
            nc.sync.dma_start(xkv_sb[:, 1:3, :], xkvr[:, 1:3, :])
            nc.sync.dma_start(
                wkz_sb[:], wkz.rearrange("(c p) h n -> p c h n", p=P))
            nc.sync.dma_start(xq_sb[:, 0:6, :], xqr[:, 0:6, :])
            nc.sync.dma_start(xkv_sb[:, 3:nkt, :], xkvr[:, 3:nkt, :])
            nc.sync.dma_start(
                wv_sb[:], wv.rearrange("(c p) n -> p c n", p=P))
            nc.sync.dma_start(
                wq_sb[:], wq.rearrange("(c p) n -> p c n", p=P))
            nc.sync.dma_start(xq_sb[:, 6:NQT, :], xqr[:, 6:NQT, :])
            for g in range(QC):
                nc.sync.dma_start(S_g[g][:, 4:4 + nkt, :],
                                  mtr[:, :, g * 512:(g + 1) * 512])
            nc.sync.dma_start(
                wo_sb[:], wo.rearrange("(c p) n -> p c n", p=P))
            if needs_bqk:
                nc.sync.dma_start(bq_sb[:], bqd)
                nc.sync.dma_start(bk_sb[:], bkd)
            st = {}
            for idx in range(len(seq) + 1):
                if idx < len(seq):
                    kind, _, t = seq[idx]
                    xt = (xkv_sb if kind == "kv" else xq_sb)[:, t, :]
                    st[idx] = ln_s1(xt)
                if idx > 0:
                    p = idx - 1
                    kind, _, t = seq[p]
                    if kind == "kv":
                        ln_s2_kv(st.pop(p), xlnkvT_g[t // 3], t % 3)
                        if t % 3 == 2:
                            kproj_group(t // 3)
                            vproj_group(t // 3)
                    else:
                        ln_s2(st.pop(p), xlnqT_g[t // 4], t % 4)
            qproj_mc(0, 0)
            qproj_mc(0, 1)

            # ---- attention: 16 blocks of (q group g, head h) ----
            pT_t = {}
            av_t = {}
            aT_t = {}
            blocks = [(g, h) for g in range(QC) for h in range(HC)]

            NCH = (nkt + 1) // 2     # score chunks of 2 kv tiles

            def exp_engine(bi, ci):
                # Pool cannot access PSUM; DVE takes the tail chunks
                # (fewer in early blocks, which carry the LN/qproj thunks)
                if (bi < 2 or bi >= 10) and ci >= NCH - 2:
                    return "dve"
                if 2 <= bi < 10 and ci == NCH - 1:
                    return "dve"
                return "act"

            ob_t = {}

            def out_tile(g, qs, alt=False, ob_eng="dve"):
                if qs == 0:
                    ob_t[g] = outp.tile([P, 4, E], bf16, tag="ob",
                                        name=f"ob{g}")
                if alt:
                    ps = psS.tile([P, 2, 512], f32, tag="sp",
                                  name="pso")[:, 0, :]
                else:
                    ps = psO.tile([P, E], f32, tag="po", name="pso")[:]
                for mc in range(MC):
                    nc.tensor.matmul(
                        ps,
                        lhsT=aT_t[g][:, mc, qs * P:(qs + 1) * P],
                        rhs=wo_sb[:, mc, :],
                        start=(mc == 0), stop=(mc == MC - 1))
                if ob_eng == "dve":
                    nc.vector.tensor_copy(ob_t[g][:, qs, :], ps)
                else:
                    nc.scalar.activation(
                        ob_t[g][:, qs, :], ps,
                        mybir.ActivationFunctionType.Identity)
                if g == QC - 1:
                    # tail: per-tile DMA so the drain overlaps the copies
                    t = g * 4 + qs
                    nc.sync.dma_start(outd[t * P:(t + 1) * P, :],
                                      ob_t[g][:, qs, :])
                elif qs == 3:
                    dst = outd[g * 4 * P:(g + 1) * 4 * P, :].rearrange(
                        "(t p) e -> p t e", p=P)
                    nc.sync.dma_start(dst, ob_t[g][:])

            def score_chunk(bi, ci):
                """Scores (+fused mask bias) + exp for chunk ci of block bi
                (2 kv tiles per chunk, 1 for the odd tail)."""
                g, h = blocks[bi]
                mc = h // 2
                pT = pT_t[bi]
                w = min(2, nkt - 2 * ci)
                sp = psS.tile([P, 2, 512], f32, tag="sp")
                for j in range(w):
                    jg = 2 * ci + j
                    s0 = jg * HC + h
                    # DR-A: k_hi.T q_hi + k_hi.T q_lo
                    nc.tensor.matmul(
                        sp[:, j, :],
                        lhsT=pair_ap(kt[:, s0, :], 0, P),
                        rhs=pair_ap(S_g[g][:, mc, :], 2, 512),
                        start=True, stop=False,
                        perf_mode=mybir.MatmulPerfMode.DoubleRow)
                    # DR-B: k_lo.T q_hi + I.T M
                    nc.tensor.matmul(
                        sp[:, j, :],
                        lhsT=pair_ap(kt[:, NS + s0, :], NS - s0, P),
                        rhs=pair_ap(S_g[g][:, mc, :], 4 + jg - mc, 512),
                        start=False, stop=True,
                        perf_mode=mybir.MatmulPerfMode.DoubleRow)
                eng = exp_engine(bi, ci)
                if eng == "act":
                    nc.scalar.activation(
                        pT[:, 2 * ci:2 * ci + w, :], sp[:, 0:w, :],
                        mybir.ActivationFunctionType.Exp,
                        scale=SCALE, bias=bact_sb[:])
                else:
                    nc.vector.tensor_scalar(
                        pT[:, 2 * ci:2 * ci + w, :].bitcast(i16),
                        sp[:, 0:w, :], A16, 0.0,
                        mybir.AluOpType.mult, mybir.AluOpType.max)

            from contextlib import contextmanager

            @contextmanager
            def low_priority(offset):
                tc.cur_priority += offset
                try:
                    yield
                finally:
                    tc.cur_priority -= offset

            # look-ahead work queue: q-side LN + qproj for groups 1-3.
            thunks = [("ln", 1, 2), ("ln", 1, 3), ("qp", 1, 0), ("qp", 1, 1)]
            for g2 in range(2, QC):
                for ti in range(4):
                    thunks.append(("ln", g2, ti))
                thunks += [("qp", g2, 0), ("qp", g2, 1)]

            def run_thunk(bi):
                if bi >= len(thunks):
                    return
                kind, g2, ti = thunks[bi]
                if kind == "ln":
                    t = g2 * 4 + ti
                    s = ln_s1(xq_sb[:, t, :])
                    ln_s2(s, xlnqT_g[g2], ti)
                else:
                    qproj_mc(g2, ti, late=True)

            for bi, (g, h) in enumerate(blocks):
                if h == 0:
                    av_t[g] = avp.tile([P, 4, HC, D], bf16, tag="av",
                                       name=f"av{g}")
                    aT_t[g] = aTp.tile([P, MC, 512], bf16, tag="aT",
                                       name=f"aT{g}")
                av = av_t[g]
                if bi not in pT_t:
                    pT_t[bi] = pTp.tile([P, nkt, 512], fp16, tag="pt",
                                        name=f"pT{bi}")
                    score_chunk(bi, 0)
                acc = psA.tile([P, 4, D + 1], f32, tag="acc")
                for ci in range(1, NCH):
                    score_chunk(bi, ci)
                with low_priority(450):
                    if bi >= 2:
                        run_thunk(2 * (bi - 2))
                        run_thunk(2 * (bi - 2) + 1)
                if bi + 1 < len(blocks):
                    pT_t[bi + 1] = pTp.tile([P, nkt, 512], fp16, tag="pt",
                                            name=f"pT{bi + 1}")
                    score_chunk(bi + 1, 0)
                # attn @ [v|1] accumulation (fp16)
                pT = pT_t[bi]
                for qs in range(4):
                    for kc in range(nkt):
                        nc.tensor.matmul(
                            acc[:, qs, :],
                            lhsT=pT[:, kc, qs * P:(qs + 1) * P],
                            rhs=vt[:, kc, h * (D + 1):(h + 1) * (D + 1)],
                            start=(kc == 0), stop=(kc == nkt - 1),
                            skip_group_check=True)
                # normalize: per-partition denominator in acc[:, :, D].
                # Every row has at least one unmasked key in this workload,
                # so the denominator is strictly positive.
                rcp = scr.tile([P, 4, 1], f32, tag="rcp")
                nc.vector.reciprocal(rcp[:, :, 0], acc[:, :, D])
                nc.vector.tensor_tensor(
                    av[:, :, h, :], acc[:, :, 0:D],
                    rcp[:].to_broadcast((P, 4, D)), mybir.AluOpType.mult)
                if h % 2 == 1:
                    pr = h // 2
                    for qs in range(4):
                        nc.tensor.transpose(
                            acc[:, qs, 0:D].bitcast(bf16),
                            av[:, qs, 2 * pr:2 * pr + 2, :], ident_bf[:])
                    nc.vector.tensor_copy(
                        aT_t[g][:, pr, :].rearrange("p (q n) -> p q n", n=P),
                        acc[:].bitcast(bf16)[:, :, 0:P])
                if g > 0:
                    with low_priority(150):
                        out_tile(g - 1, h, ob_eng="dve" if bi < 8 else "act")
            for qs in range(4):
                out_tile(QC - 1, qs, alt=(qs % 2 == 1),
                         ob_eng="act" if qs % 2 else "dve")

    return nc


def _get_nc(needs_bv: bool = False, reps: int = 1, nkt: int | None = None,
            needs_bqk: bool | None = None):
    global _LAST_KEY
    if nkt is None:
        if _LAST_KEY is not None:
            return _CACHE[_LAST_KEY]
        nkt = 9
    if needs_bqk is None:
        needs_bqk = needs_bv
    key = ("nc", nkt, needs_bqk, needs_bv)
    if key not in _CACHE:
        _CACHE[key] = _build(nkt, needs_bqk, needs_bv)
    _LAST_KEY = key
    return _CACHE[key]


def kernel(query, key_value, kv_mask, sparse_mask,
           ln_q_g, ln_q_b, ln_kv_g, ln_kv_b,
           Wq, bq, Wk, bk, Wv, bv, Wo, bo):
    query = np.asarray(query, np.float32)
    key_value = np.asarray(key_value, np.float32)
    kv_mask = np.asarray(kv_mask)
    sparse_mask = np.asarray(sparse_mask)
    B = query.shape[0]

    # Fold LN gain/bias into the projection weights (exact algebra):
    # (x_ln*g + b) @ W + c  ==  x_ln @ (g[:,None]*W) + (b@W + c)
    Wq_g = np.asarray(ln_q_g, np.float32)[:, None] * np.asarray(Wq, np.float32)
    Wk_g = np.asarray(ln_kv_g, np.float32)[:, None] * np.asarray(Wk, np.float32)
    Wv_g = np.asarray(ln_kv_g, np.float32)[:, None] * np.asarray(Wv, np.float32)
    bq_e = np.asarray(ln_q_b, np.float32) @ np.asarray(Wq, np.float32) + bq
    bk_e = np.asarray(ln_kv_b, np.float32) @ np.asarray(Wk, np.float32) + bk
    bv_e = np.asarray(ln_kv_b, np.float32) @ np.asarray(Wv, np.float32) + bv

    needs_bqk = bool(np.any(bq_e != 0.0) or np.any(bk_e != 0.0))
    needs_bv = bool(np.any(bv_e != 0.0))

    # Compact the kv sequence: tokens with kv_mask=0 are masked for every
    # query, so drop them and pad to a multiple of 384 (3 kv tiles).
    valid = [np.flatnonzero(kv_mask[b]) for b in range(B)]
    nv_max = max(1, max(len(v) for v in valid))
    nkt = 3 * math.ceil(math.ceil(nv_max / P) / 3)
    KT = nkt * P

    nc = _get_nc(needs_bv, nkt=nkt, needs_bqk=needs_bqk)

    xkvc = np.zeros((B, KT, E), np.float32)
    mtc = np.full((B, KT, T), MASK_OFF, F8)
    on8 = F8(MASK_ON)
    off8 = F8(MASK_OFF)
    for b in range(B):
        nv = len(valid[b])
        xkvc[b, :nv] = key_value[b][valid[b]]
        mtc[b, :nv] = np.where(sparse_mask[b].T[valid[b]], on8, off8)

    in_maps = []
    for c in range(8):
        b, hg = c // 2, c % 2
        hs = slice(hg * MC * P, (hg + 1) * MC * P)
        wk_c = np.asarray(Wk_g[:, hs], np.float32)     # [E, 256]
        wkz = np.zeros((E, HC, P), np.float32)
        for h in range(HC):
            po = (h % 2) * D
            wkz[:, h, po:po + D] = wk_c[:, h * D:(h + 1) * D]
        m = {
            "xq": np.ascontiguousarray(query[b]).astype(BF16),
            "xkv": np.ascontiguousarray(xkvc[b]).astype(BF16),
            "wq": np.ascontiguousarray(Wq_g[:, hs]).astype(BF16),
            "wkz": wkz.astype(BF16),
            "wv": np.ascontiguousarray(Wv_g[:, hs]).astype(BF16),
            "wo": np.ascontiguousarray(
                np.asarray(Wo, np.float32)[hs, :]).astype(BF16),
            "mt": np.ascontiguousarray(mtc[b]),
        }
        if needs_bqk:
            m["bq"] = np.ascontiguousarray(
                bq_e[hs].reshape(MC, P).T.astype(np.float32))
            bk_c = bk_e[hs].reshape(HC, D)
            bkz = np.zeros((HC, P), np.float32)
            for h in range(HC):
                po = (h % 2) * D
                bkz[h, po:po + D] = bk_c[h]
            m["bk"] = bkz.reshape(1, HC * P).astype(BF16)
        if needs_bv:
            m["bv"] = bv_e[hs].astype(BF16).reshape(1, MC * P)
        in_maps.append(m)

    if not getattr(nc, "_sync_waits_split", False):
        _split_sync_waits(nc)
        nc._sync_waits_split = True
    res = bass_utils.run_bass_kernel_spmd(
        nc, in_maps, core_ids=list(range(8)),
        trace=bool(os.environ.get("KERNEL_TRACE")))
    globals()["LAST_RESULTS"] = res

    bo_f = np.asarray(bo, np.float32)
    out = np.empty((B, T, E), np.float32)
    for b in range(B):
        out[b] = (res.results[2 * b]["out"].astype(np.float32)
                  + res.results[2 * b + 1]["out"].astype(np.float32) + bo_f)
    return out


# revision 46
# speedup vs baseline: 1.0375x; 1.0375x over previous
"""Trainium2 Bass kernel for nn_CrossAttentionLayer (sparse cross attention).

Sharding: 8 cores = 4 batches x 2 head-groups. Core c handles batch c//2 and
heads [4*(c%2), 4*(c%2)+4). Host compacts the kv sequence per batch using
kv_mask, padding to a multiple of 384 tokens.

v3 design (cost-model-driven):
  - Scores run as TWO fp8 DoubleRow matmuls per (kv-tile, 512q) tile with a
    hi/lo split for full bf16-level precision at half the bf16 row cost:
      DR-A: k_hi.T q_hi + k_hi.T q_lo   (stride-0 weight slot pair)
      DR-B: k_lo.T q_hi + I.T M
    where x = fp8(x_hi) + fp8(x_lo) reconstructs x to ~0.2% and M holds the
    sparse-mask bias {+112 valid, -128 masked} plus the softmax range shift.
    The k_lo*q_lo term (~0.1%) is dropped.  No elementwise mask work exists
    anywhere.
  - p tiles and v are fp16.  exp splits across three engines: ACT (true exp,
    bias matching the Schraudolph scale), DVE and Pool (Schraudolph:
    int16 bits = floor(max(128*log2e*(s+112), 0)) bitcast fp16; masked
    entries clamp to exactly +0.0).
  - q/k/v/out projections and attn@V stay bf16/fp16 for accuracy.
  - All layernorm transposes go through DMA-transpose (no PSUM round trip);
    LN applies use DVE's 4x mode.  kproj uses per-head zero-interleaved
    weights so kT hi/lo slots get PE-computed zero padding in the
    complementary head half.
"""

import math
import os

import numpy as np
import ml_dtypes

import bass_rust
import concourse.bass as bass
import concourse.mybir as mybir
import concourse.tile as tile
from concourse import bass_utils
from concourse.ap import AP
from concourse.masks import make_identity
from concourse.vector_clock import ScopedClock


class _TileContext(tile.TileContext):
    """TileContext whose kernel-tail drain is split into single-wait drains.

    The walrus build in this environment rejects >1 sync-wait on a Drain
    (CTRL_NO struct): "Too many sync wait commands". The stock
    _drain_and_barrier attaches one wait per outstanding semaphore to a
    single Drain; emit one Drain per wait instead.
    """

    def _drain_and_barrier(self, tick_clock, wait_clock):
        drain_inst = self.nc.sync.drain()
        wait_clock.add_sem_waits(
            drain_inst.ins, ScopedClock({None: tick_clock.global_clock})
        )
        si = drain_inst.ins.sync_info
        if si is not None and si.on_wait and len(si.on_wait) > 1:
            waits = list(si.on_wait)
            drain_inst.ins.sync_info = bass_rust.SyncInfo(
                on_wait=[waits[0]], on_update=si.on_update or [])
            for w in waits[1:]:
                extra = self.nc.sync.drain()
                extra.ins.sync_info = bass_rust.SyncInfo(
                    on_wait=[w], on_update=[])

        self.nc.all_engine_barrier()
        assert self.sems is not None
        popped = self.nc._tile_sem_poison_stack.pop()
        assert popped is self._sem_poison
        self.nc.clear_and_free_semaphores(list(self.sems.allocated().values()))
        self.nc.all_engine_barrier()


def _split_sync_waits(nc):
    """Cap every instruction at one sync wait (walrus build limitation)."""
    for f in nc.m.functions:
        for bb in f.blocks:
            insns = bb.instructions
            out = []
            changed = False
            for ins in insns:
                si = ins.sync_info
                if si is not None and si.on_wait and len(si.on_wait) > 1:
                    waits = list(si.on_wait)
                    for w in waits[:-1]:
                        nop = mybir.InstNoOp(
                            name=nc.get_next_instruction_name(),
                            engine=ins.engine,
                            ins=[], outs=[],
                            sync_info=bass_rust.SyncInfo(
                                on_wait=[w], on_update=[]),
                        )
                        out.append(nop)
                    ins.sync_info = bass_rust.SyncInfo(
                        on_wait=[waits[-1]], on_update=si.on_update or [])
                    changed = True
                out.append(ins)
            if changed:
                bb.instructions = out


BF16 = ml_dtypes.bfloat16
F8 = ml_dtypes.float8_e4m3
FP16 = np.float16

E = 512
H = 8
D = 64
T = 2048           # query tokens
P = 128
NQT = T // P       # 16 query token tiles
EC = E // P        # 4 contraction chunks
HC = 4             # heads per core
MC = 2             # 128-wide chunks of this core's 256 head dims
QC = 4             # 512-wide query chunks
SCALE = float(D) ** -0.5
EPS = 1e-5
MASK_ON = 112.0      # valid-entry score bias (fp8-exact)
MASK_OFF = -128.0    # masked-entry score bias (fp8-exact)
# Schraudolph exp in fp16: bits = A16*(s + 112), value = 2^((bits-15360)/1024)
A16 = 184.66496280150618          # 128 * log2(e)
# ACT path must match the Schraudolph path's MEAN scale: the Schraudolph
# chord (1+f)/2^f overestimates by x1.04068 on average, so the ACT path is
# scaled up by the same factor (uniform across chunks -> cancels in softmax):
# exp(SCALE*(s+112) + BIAS_ACT) = 1.04068 * 2^((A16*(s+112) - 15360)/1024)
BIAS_ACT = -10.35732923219427    # (A16*112-15360)*ln2/1024 - 14 + ln(1.04068)

_CACHE = {}
_LAST_KEY = None


def _build(nkt: int, needs_bqk: bool, needs_bv: bool):
    assert nkt % 3 == 0, "kv tiles padded to a multiple of 3"
    nkg = nkt // 3               # kv groups of 3 tiles / score chunks
    KT = nkt * P
    NS = nkt * HC                # hi slots; lo slots at NS+s; identity at 2NS

    nc = bass.Bass("TRN2", target_bir_lowering=False, debug=False,
                   num_devices=8)
    f32 = mybir.dt.float32
    bf16 = mybir.dt.bfloat16
    fp16 = mybir.dt.float16
    fp8 = mybir.dt.float8e4
    i16 = mybir.dt.int16

    xq = nc.dram_tensor("xq", [T, E], bf16, kind="ExternalInput").ap()
    xkv = nc.dram_tensor("xkv", [KT, E], bf16, kind="ExternalInput").ap()
    wq = nc.dram_tensor("wq", [E, MC * P], bf16, kind="ExternalInput").ap()
    # zero-interleaved per-head k weights: [E, HC, P] with head h occupying
    # columns (h%2)*64..(h%2)*64+64, zeros elsewhere
    wkz = nc.dram_tensor("wkz", [E, HC, P], bf16, kind="ExternalInput").ap()
    wv = nc.dram_tensor("wv", [E, MC * P], bf16, kind="ExternalInput").ap()
    wo = nc.dram_tensor("wo", [MC * P, E], bf16, kind="ExternalInput").ap()
    mtd = nc.dram_tensor("mt", [KT, T], fp8, kind="ExternalInput").ap()
    if needs_bqk:
        bqd = nc.dram_tensor("bq", [P, MC], f32, kind="ExternalInput").ap()
        bkd = nc.dram_tensor("bk", [1, HC * P], bf16,
                             kind="ExternalInput").ap()
    if needs_bv:
        bvd = nc.dram_tensor("bv", [1, MC * P], bf16, kind="ExternalInput").ap()
    outd = nc.dram_tensor("out", [T, E], bf16, kind="ExternalOutput").ap()

    mtr = mtd.rearrange("(c p) q -> p c q", p=P)

    with _TileContext(nc) as tc:
        with (
            tc.tile_pool(name="persist", bufs=1) as pp,
            tc.tile_pool(name="xln", bufs=4) as xlnp,
            tc.tile_pool(name="scratch", bufs=4) as scr,
            tc.tile_pool(name="ostage", bufs=2) as outp,
            tc.tile_pool(name="pt", bufs=2) as pTp,
            tc.tile_pool(name="av", bufs=2) as avp,
            tc.tile_pool(name="at", bufs=2) as aTp,
            tc.tile_pool(name="psS", bufs=3, space="PSUM") as psS,
            tc.tile_pool(name="psA", bufs=1, space="PSUM") as psA,
            tc.tile_pool(name="psO", bufs=1, space="PSUM") as psO,
        ):
            # ---- persistent SBUF tensors ----
            xq_sb = pp.tile([P, NQT, E], bf16, tag="xqs")
            xkv_sb = pp.tile([P, nkt, E], bf16, tag="xkvs")
            wq_sb = pp.tile([P, EC, MC * P], bf16, tag="wq")
            wkz_sb = pp.tile([P, EC, HC, P], bf16, tag="wkz")
            wv_sb = pp.tile([P, EC, MC * P], bf16, tag="wv")
            wo_sb = pp.tile([P, MC, E], bf16, tag="wo")
            # kT: hi slots [0, NS), lo slots [NS, 2NS), identity at 2NS
            kt = pp.tile([P, 2 * NS + 1, P], fp8, tag="kt")
            # v: nkt tiles x HC heads x (D + ones-col), fp16
            vt = pp.tile([P, nkt, HC * (D + 1)], fp16, tag="vt")
            # scores rhs: slots [q_hi mc0, q_hi mc1, q_lo mc0, q_lo mc1,
            #                    M_0..M_{nkt-1}]
            S_g = [pp.tile([P, 4 + nkt, 512], fp8, tag=f"S{g}", name=f"S{g}")
                   for g in range(QC)]
            xlnkvT_g = [pp.tile([P, EC, 3 * P], bf16, tag=f"xlnkv{i}",
                                name=f"xlnkv{i}") for i in range(nkg)]
            xlnqT_g = [pp.tile([P, EC, 4 * P], bf16, tag=f"xlnq{g}",
                               name=f"xlnq{g}") for g in range(QC)]
            if needs_bqk:
                bq_sb = pp.tile([P, MC], f32, tag="bq")
                bk_sb = pp.tile([1, HC * P], bf16, tag="bk")
            if needs_bv:
                bv_sb = pp.tile([1, MC * P], bf16, tag="bv")
            if needs_bqk or needs_bv:
                ones1 = pp.tile([1, E], bf16, tag="ones1")
                nc.vector.memset(ones1[:], 1.0)

            eps_sb = pp.tile([P, 1], f32, tag="eps")
            nc.vector.memset(eps_sb[:], EPS)
            bact_sb = pp.tile([P, 1], f32, tag="bact")
            nc.vector.memset(bact_sb[:], BIAS_ACT)
            if needs_bv:
                nc.sync.dma_start(bv_sb[:], bvd)
            make_identity(nc, kt[:, 2 * NS, :])
            ident_bf = pp.tile([P, P], bf16, tag="identbf")
            make_identity(nc, ident_bf[:])
            # ones columns of v (per head, per kv tile)
            vt4 = vt[:].rearrange("p k (h d) -> p k h d", d=D + 1)
            nc.gpsimd.memset(vt4[:, :, :, D], 1.0)

            def pair_ap(base, stride_slots, width):
                """[P, 2, width] AP: slot pair {base, base+stride}."""
                return AP(base.tensor, base.offset,
                          [list(base.ap[0]), [stride_slots * width, 2],
                           [1, width]])

            def ln_s1(xt):
                """LN stage 1: stats + sqrt on a staged [P, E] slice."""
                stats = scr.tile([P, 6], f32, tag="bnstats")
                mv = scr.tile([P, 2], f32, tag="bnmv")
                nc.vector.bn_stats(stats[:], xt)
                nc.vector.bn_aggr(mv[:], stats[:])
                sig = scr.tile([P, 1], f32, tag="sig")
                nc.scalar.activation(
                    sig[:], mv[:, 1:2],
                    mybir.ActivationFunctionType.Sqrt, bias=eps_sb[:])
                return xt, mv, sig

            def ln_apply(st, pool=False):
                xt, mv, sig = st
                rsig = scr.tile([P, 1], f32, tag="rsig")
                nc.vector.reciprocal(rsig[:], sig[:])
                xln = xlnp.tile([P, E], bf16, tag="xln")
                if pool:
                    nc.gpsimd.tensor_scalar(
                        xln[:], xt, mv[:, 0:1], rsig[:],
                        mybir.AluOpType.subtract, mybir.AluOpType.mult)
                else:
                    nc.vector.tensor_scalar(
                        xln[:], xt, mv[:, 0:1], rsig[:],
                        mybir.AluOpType.subtract, mybir.AluOpType.mult)
                return xln

            def ln_s2(st, dstT, ti, pool=False):
                """q LN stage 2: normalize + SP DMA transpose."""
                xln = ln_apply(st, pool)
                nc.sync.dma_start_transpose(
                    dstT[:, :, ti * P:(ti + 1) * P], xln[:])

            def ln_s2_kv(st, dstT, ti):
                """kv LN stage 2: normalize + PE transpose + ACT copy."""
                xln = ln_apply(st)
                tps = psA.tile([P, 4, D + 1], f32, tag="acc")
                tp = tps[:].rearrange("p a b -> p (a b)").bitcast(bf16)
                for c in range(EC):
                    nc.tensor.transpose(
                        tp[:, c * P:(c + 1) * P], xln[:, c * P:(c + 1) * P],
                        ident_bf[:])
                nc.scalar.activation(
                    dstT[:, :, ti * P:(ti + 1) * P],
                    tp[:, 0:E].rearrange("p (c n) -> p c n", n=P),
                    mybir.ActivationFunctionType.Identity)

            def kproj_group(i):
                """Per-head k projection (zero-interleaved weights) + hi/lo
                fp8 writes into kt slots."""
                for h in range(HC):
                    ps = psS.tile([P, 2, 512], f32, tag="sp")
                    pk = ps[:].rearrange("p a b -> p (a b)")[:, 0:3 * P]
                    for c in range(EC):
                        nc.tensor.matmul(
                            pk,
                            lhsT=wkz_sb[:, c, h, :],
                            rhs=xlnkvT_g[i][:, c, :],
                            start=(c == 0),
                            stop=(c == EC - 1 and not needs_bqk))
                    if needs_bqk:
                        nc.tensor.matmul(
                            pk, lhsT=bk_sb[:, h * P:(h + 1) * P],
                            rhs=ones1[:, 0:3 * P], start=False, stop=True)
                    s0 = i * 3 * HC + h
                    bh = kt[:, s0, :]
                    bl = kt[:, NS + s0, :]
                    hi3 = AP(bh.tensor, bh.offset,
                             [list(bh.ap[0]), [HC * P, 3], [1, P]])
                    lo3 = AP(bl.tensor, bl.offset,
                             [list(bl.ap[0]), [HC * P, 3], [1, P]])
                    # ACT stages psum->bf16 (frees the PSUM buf fast);
                    # DVE/Pool split hi = fp8(k), lo = fp8(k - hi) in SBUF.
                    kstg = scr.tile([P, 3, P], bf16, tag="kstg")
                    nc.scalar.activation(
                        kstg[:], pk.rearrange("p (a n) -> p a n", n=P),
                        mybir.ActivationFunctionType.Identity)
                    nc.vector.tensor_copy(hi3, kstg[:])
                    nc.gpsimd.tensor_tensor(
                        lo3, kstg[:], hi3, mybir.AluOpType.subtract)

            def vproj_group(i):
                for ti in range(3):
                    ps = psS.tile([P, 2, 512], f32, tag="sp")
                    pv = ps[:].rearrange("p a b -> p (a b)")[:, 0:HC * D]
                    for c in range(EC):
                        nc.tensor.matmul(
                            pv,
                            lhsT=xlnkvT_g[i][:, c, ti * P:(ti + 1) * P],
                            rhs=wv_sb[:, c, :],
                            start=(c == 0),
                            stop=(c == EC - 1 and not needs_bv))
                    if needs_bv:
                        nc.tensor.matmul(
                            pv, lhsT=ones1[:, 0:P], rhs=bv_sb[:],
                            start=False, stop=True)
                    vd = vt[:, 3 * i + ti].rearrange(
                        "p (h d) -> p h d", d=D + 1)
                    nc.scalar.activation(
                        vd[:, :, 0:D],
                        pv.rearrange("p (h d) -> p h d", d=D),
                        mybir.ActivationFunctionType.Identity)

            def qproj_mc(g, mc, late=False):
                if late:
                    psq = psO.tile([P, E], f32, tag="po", name="psq")[:]
                else:
                    psq = psS.tile([P, 2, 512], f32, tag="sp",
                                   name="psq")[:, 0, :]
                for c in range(EC):
                    nc.tensor.matmul(
                        psq,
                        lhsT=wq_sb[:, c, mc * P:(mc + 1) * P],
                        rhs=xlnqT_g[g][:, c, :],
                        start=(c == 0), stop=(c == EC - 1))
                if late and not needs_bqk:
                    # block phase: DVE stages, Pool (idle there) splits
                    qstg = scr.tile([P, E], bf16, tag="qstg")
                    nc.vector.tensor_copy(qstg[:], psq)
                    nc.gpsimd.tensor_copy(S_g[g][:, mc, :], qstg[:])
                    nc.gpsimd.tensor_tensor(
                        S_g[g][:, 2 + mc, :], qstg[:], S_g[g][:, mc, :],
                        mybir.AluOpType.subtract)
                else:
                    # prologue: ACT stages psum->bf16 (with bias), then
                    # DVE hi + Pool lo split in SBUF
                    qstg = scr.tile([P, E], bf16, tag="qstg")
                    nc.scalar.activation(
                        qstg[:], psq,
                        mybir.ActivationFunctionType.Identity,
                        bias=bq_sb[:, mc:mc + 1] if needs_bqk else 0.0)
                    nc.vector.tensor_copy(S_g[g][:, mc, :], qstg[:])
                    nc.gpsimd.tensor_tensor(
                        S_g[g][:, 2 + mc, :], qstg[:], S_g[g][:, mc, :],
                        mybir.AluOpType.subtract)

            # ---- prologue: kv side then q groups 0-1, 2-stage pipelined ----
            kv_entries = [("kv", xkv, t) for t in range(3 * nkg)]
            q_entries = [("q", xq, t) for t in range(6)]
            seq = []
            qi = 0
            for t in range(3 * nkg):
                seq.append(kv_entries[t])
                if t % 3 == 2 and qi < 2:
                    seq.append(q_entries[qi])
                    qi += 1
            seq += q_entries[qi:]
            xkvr = xkv.rearrange("(t p) e -> p t e", p=P)
            xqr = xq.rearrange("(t p) e -> p t e", p=P)
            nc.sync.dma_start(xkv_sb[:, 0:1, :], xkvr[:, 0:1, :])
            nc.sync.dma_start(xkv_sb[:, 1:3, :], xkvr[:, 1:3, :])
            nc.sync.dma_start(
                wkz_sb[:], wkz.rearrange("(c p) h n -> p c h n", p=P))
            nc.sync.dma_start(xq_sb[:, 0:6, :], xqr[:, 0:6, :])
            nc.sync.dma_start(xkv_sb[:, 3:nkt, :], xkvr[:, 3:nkt, :])
            nc.sync.dma_start(
                wv_sb[:], wv.rearrange("(c p) n -> p c n", p=P))
            nc.sync.dma_start(
                wq_sb[:], wq.rearrange("(c p) n -> p c n", p=P))
            nc.sync.dma_start(xq_sb[:, 6:NQT, :], xqr[:, 6:NQT, :])
            for g in range(QC):
                nc.sync.dma_start(S_g[g][:, 4:4 + nkt, :],
                                  mtr[:, :, g * 512:(g + 1) * 512])
            nc.sync.dma_start(
                wo_sb[:], wo.rearrange("(c p) n -> p c n", p=P))
            if needs_bqk:
                nc.sync.dma_start(bq_sb[:], bqd)
                nc.sync.dma_start(bk_sb[:], bkd)
            st = {}
            for idx in range(len(seq) + 1):
                if idx < len(seq):
                    kind, _, t = seq[idx]
                    xt = (xkv_sb if kind == "kv" else xq_sb)[:, t, :]
                    st[idx] = ln_s1(xt)
                if idx > 0:
                    p = idx - 1
                    kind, _, t = seq[p]
                    if kind == "kv":
                        ln_s2_kv(st.pop(p), xlnkvT_g[t // 3], t % 3)
                        if t % 3 == 2:
                            kproj_group(t // 3)
                            vproj_group(t // 3)
                    else:
                        ln_s2(st.pop(p), xlnqT_g[t // 4], t % 4)
            qproj_mc(0, 0)
            qproj_mc(0, 1)

            # ---- attention: 16 blocks of (q group g, head h) ----
            pT_t = {}
            av_t = {}
            aT_t = {}
            blocks = [(g, h) for g in range(QC) for h in range(HC)]

            NCH = (nkt + 1) // 2     # score chunks of 2 kv tiles

            def exp_engine(bi, ci):
                # Pool cannot access PSUM; DVE takes the tail chunks
                # (fewer in early blocks, which carry the LN/qproj thunks)
                if bi >= 8 and ci >= NCH - 2:
                    return "dve"
                if 5 <= bi < 8 and ci >= NCH - 2:
                    return "dve"
                if bi < 5 and ci == NCH - 1:
                    return "dve"
                return "act"

            ob_t = {}

            def out_tile(g, qs, alt=False, ob_eng="dve"):
                if qs == 0:
                    ob_t[g] = outp.tile([P, 4, E], bf16, tag="ob",
                                        name=f"ob{g}")
                if alt:
                    ps = psS.tile([P, 2, 512], f32, tag="sp",
                                  name="pso")[:, 0, :]
                else:
                    ps = psO.tile([P, E], f32, tag="po", name="pso")[:]
                for mc in range(MC):
                    nc.tensor.matmul(
                        ps,
                        lhsT=aT_t[g][:, mc, qs * P:(qs + 1) * P],
                        rhs=wo_sb[:, mc, :],
                        start=(mc == 0), stop=(mc == MC - 1))
                if ob_eng == "dve":
                    nc.vector.tensor_copy(ob_t[g][:, qs, :], ps)
                else:
                    nc.scalar.activation(
                        ob_t[g][:, qs, :], ps,
                        mybir.ActivationFunctionType.Identity)
                if g == QC - 1:
                    # tail: per-tile DMA so the drain overlaps the copies
                    t = g * 4 + qs
                    nc.sync.dma_start(outd[t * P:(t + 1) * P, :],
                                      ob_t[g][:, qs, :])
                elif qs == 3:
                    dst = outd[g * 4 * P:(g + 1) * 4 * P, :].rearrange(
                        "(t p) e -> p t e", p=P)
                    nc.sync.dma_start(dst, ob_t[g][:])

            def score_chunk(bi, ci):
                """Scores (+fused mask bias) + exp for chunk ci of block bi
                (2 kv tiles per chunk, 1 for the odd tail)."""
                g, h = blocks[bi]
                mc = h // 2
                pT = pT_t[bi]
                w = min(2, nkt - 2 * ci)
                sp = psS.tile([P, 2, 512], f32, tag="sp")
                for j in range(w):
                    jg = 2 * ci + j
                    s0 = jg * HC + h
                    # DR-A: k_hi.T q_hi + k_hi.T q_lo
                    nc.tensor.matmul(
                        sp[:, j, :],
                        lhsT=pair_ap(kt[:, s0, :], 0, P),
                        rhs=pair_ap(S_g[g][:, mc, :], 2, 512),
                        start=True, stop=False,
                        perf_mode=mybir.MatmulPerfMode.DoubleRow)
                    # DR-B: k_lo.T q_hi + I.T M
                    nc.tensor.matmul(
                        sp[:, j, :],
                        lhsT=pair_ap(kt[:, NS + s0, :], NS - s0, P),
                        rhs=pair_ap(S_g[g][:, mc, :], 4 + jg - mc, 512),
                        start=False, stop=True,
                        perf_mode=mybir.MatmulPerfMode.DoubleRow)
                eng = exp_engine(bi, ci)
                if eng == "act":
                    nc.scalar.activation(
                        pT[:, 2 * ci:2 * ci + w, :], sp[:, 0:w, :],
                        mybir.ActivationFunctionType.Exp,
                        scale=SCALE, bias=bact_sb[:])
                else:
                    nc.vector.tensor_scalar(
                        pT[:, 2 * ci:2 * ci + w, :].bitcast(i16),
                        sp[:, 0:w, :], A16, 0.0,
                        mybir.AluOpType.mult, mybir.AluOpType.max)

            from contextlib import contextmanager

            @contextmanager
            def low_priority(offset):
                tc.cur_priority += offset
                try:
                    yield
                finally:
                    tc.cur_priority -= offset

            # look-ahead work queue: q-side LN + qproj for groups 1-3.
            thunks = [("ln", 1, 2), ("ln", 1, 3), ("qp", 1, 0), ("qp", 1, 1)]
            for g2 in range(2, QC):
                for ti in range(4):
                    thunks.append(("ln", g2, ti))
                thunks += [("qp", g2, 0), ("qp", g2, 1)]

            def run_thunk(bi):
                if bi >= len(thunks):
                    return
                kind, g2, ti = thunks[bi]
                if kind == "ln":
                    t = g2 * 4 + ti
                    s = ln_s1(xq_sb[:, t, :])
                    ln_s2(s, xlnqT_g[g2], ti)
                else:
                    qproj_mc(g2, ti, late=True)

            for bi, (g, h) in enumerate(blocks):
                if h == 0:
                    av_t[g] = avp.tile([P, 4, HC, D], bf16, tag="av",
                                       name=f"av{g}")
                    aT_t[g] = aTp.tile([P, MC, 512], bf16, tag="aT",
                                       name=f"aT{g}")
                av = av_t[g]
                if bi not in pT_t:
                    pT_t[bi] = pTp.tile([P, nkt, 512], fp16, tag="pt",
                                        name=f"pT{bi}")
                    score_chunk(bi, 0)
                acc = psA.tile([P, 4, D + 1], f32, tag="acc")
                for ci in range(1, NCH):
                    score_chunk(bi, ci)
                with low_priority(450):
                    if bi >= 1:
                        run_thunk(2 * (bi - 1))
                        run_thunk(2 * (bi - 1) + 1)
                if bi + 1 < len(blocks):
                    pT_t[bi + 1] = pTp.tile([P, nkt, 512], fp16, tag="pt",
                                            name=f"pT{bi + 1}")
                    score_chunk(bi + 1, 0)
                # attn @ [v|1] accumulation (fp16)
                pT = pT_t[bi]
                for qs in range(4):
                    for kc in range(nkt):
                        nc.tensor.matmul(
                            acc[:, qs, :],
                            lhsT=pT[:, kc, qs * P:(qs + 1) * P],
                            rhs=vt[:, kc, h * (D + 1):(h + 1) * (D + 1)],
                            start=(kc == 0), stop=(kc == nkt - 1),
                            skip_group_check=True)
                # normalize: per-partition denominator in acc[:, :, D].
                # Every row has at least one unmasked key in this workload,
                # so the denominator is strictly positive.
                rcp = scr.tile([P, 4, 1], f32, tag="rcp")
                nc.vector.reciprocal(rcp[:, :, 0], acc[:, :, D])
                nc.vector.tensor_tensor(
                    av[:, :, h, :], acc[:, :, 0:D],
                    rcp[:].to_broadcast((P, 4, D)), mybir.AluOpType.mult)
                if h % 2 == 1:
                    pr = h // 2
                    for qs in range(4):
                        nc.tensor.transpose(
                            acc[:, qs, 0:D].bitcast(bf16),
                            av[:, qs, 2 * pr:2 * pr + 2, :], ident_bf[:])
                    nc.vector.tensor_copy(
                        aT_t[g][:, pr, :].rearrange("p (q n) -> p q n", n=P),
                        acc[:].bitcast(bf16)[:, :, 0:P])
                if g > 0:
                    with low_priority(150):
                        out_tile(g - 1, h, ob_eng="dve" if bi < 8 else "act")
            for qs in range(4):
                out_tile(QC - 1, qs, alt=(qs % 2 == 1),
                         ob_eng="act" if qs % 2 else "dve")

    return nc


def _get_nc(needs_bv: bool = False, reps: int = 1, nkt: int | None = None,
            needs_bqk: bool | None = None):
    global _LAST_KEY
    if nkt is None:
        if _LAST_KEY is not None:
            return _CACHE[_LAST_KEY]
        nkt = 9
    if needs_bqk is None:
        needs_bqk = needs_bv
    key = ("nc", nkt, needs_bqk, needs_bv)
    if key not in _CACHE:
        _CACHE[key] = _build(nkt, needs_bqk, needs_bv)
    _LAST_KEY = key
    return _CACHE[key]


def kernel(query, key_value, kv_mask, sparse_mask,
           ln_q_g, ln_q_b, ln_kv_g, ln_kv_b,
           Wq, bq, Wk, bk, Wv, bv, Wo, bo):
    query = np.asarray(query, np.float32)
    key_value = np.asarray(key_value, np.float32)
    kv_mask = np.asarray(kv_mask)
    sparse_mask = np.asarray(sparse_mask)
    B = query.shape[0]

    # Fold LN gain/bias into the projection weights (exact algebra):
    # (x_ln*g + b) @ W + c  ==  x_ln @ (g[:,None]*W) + (b@W + c)
    Wq_g = np.asarray(ln_q_g, np.float32)[:, None] * np.asarray(Wq, np.float32)
    Wk_g = np.asarray(ln_kv_g, np.float32)[:, None] * np.asarray(Wk, np.float32)
    Wv_g = np.asarray(ln_kv_g, np.float32)[:, None] * np.asarray(Wv, np.float32)
    bq_e = np.asarray(ln_q_b, np.float32) @ np.asarray(Wq, np.float32) + bq
    bk_e = np.asarray(ln_kv_b, np.float32) @ np.asarray(Wk, np.float32) + bk
    bv_e = np.asarray(ln_kv_b, np.float32) @ np.asarray(Wv, np.float32) + bv

    needs_bqk = bool(np.any(bq_e != 0.0) or np.any(bk_e != 0.0))
    needs_bv = bool(np.any(bv_e != 0.0))

    # Compact the kv sequence: tokens with kv_mask=0 are masked for every
    # query, so drop them and pad to a multiple of 384 (3 kv tiles).
    valid = [np.flatnonzero(kv_mask[b]) for b in range(B)]
    nv_max = max(1, max(len(v) for v in valid))
    nkt = 3 * math.ceil(math.ceil(nv_max / P) / 3)
    KT = nkt * P

    nc = _get_nc(needs_bv, nkt=nkt, needs_bqk=needs_bqk)

    xkvc = np.zeros((B, KT, E), np.float32)
    mtc = np.full((B, KT, T), MASK_OFF, F8)
    on8 = F8(MASK_ON)
    off8 = F8(MASK_OFF)
    for b in range(B):
        nv = len(valid[b])
        xkvc[b, :nv] = key_value[b][valid[b]]
        mtc[b, :nv] = np.where(sparse_mask[b].T[valid[b]], on8, off8)

    in_maps = []
    for c in range(8):
        b, hg = c // 2, c % 2
        hs = slice(hg * MC * P, (hg + 1) * MC * P)
        wk_c = np.asarray(Wk_g[:, hs], np.float32)     # [E, 256]
        wkz = np.zeros((E, HC, P), np.float32)
        for h in range(HC):
            po = (h % 2) * D
            wkz[:, h, po:po + D] = wk_c[:, h * D:(h + 1) * D]
        m = {
            "xq": np.ascontiguousarray(query[b]).astype(BF16),
            "xkv": np.ascontiguousarray(xkvc[b]).astype(BF16),
            "wq": np.ascontiguousarray(Wq_g[:, hs]).astype(BF16),
            "wkz": wkz.astype(BF16),
            "wv": np.ascontiguousarray(Wv_g[:, hs]).astype(BF16),
            "wo": np.ascontiguousarray(
                np.asarray(Wo, np.float32)[hs, :]).astype(BF16),
            "mt": np.ascontiguousarray(mtc[b]),
        }
        if needs_bqk:
            m["bq"] = np.ascontiguousarray(
                bq_e[hs].reshape(MC, P).T.astype(np.float32))
            bk_c = bk_e[hs].reshape(HC, D)
            bkz = np.zeros((HC, P), np.float32)
            for h in range(HC):
                po = (h % 2) * D
                bkz[h, po:po + D] = bk_c[h]
            m["bk"] = bkz.reshape(1, HC * P).astype(BF16)
        if needs_bv:
            m["bv"] = bv_e[hs].astype(BF16).reshape(1, MC * P)
        in_maps.append(m)

    if not getattr(nc, "_sync_waits_split", False):
        _split_sync_waits(nc)
        nc._sync_waits_split = True
    res = bass_utils.run_bass_kernel_spmd(
        nc, in_maps, core_ids=list(range(8)),
        trace=bool(os.environ.get("KERNEL_TRACE")))
    globals()["LAST_RESULTS"] = res

    bo_f = np.asarray(bo, np.float32)
    out = np.empty((B, T, E), np.float32)
    for b in range(B):
        out[b] = (res.results[2 * b]["out"].astype(np.float32)
                  + res.results[2 * b + 1]["out"].astype(np.float32) + bo_f)
    return out
